# revision 22
# baseline (speedup 1.0000x reference)
"""Trainium2 Bass kernel for nn_CrossDConv (dense_cnn).

Math (per batch sample b, see reference):
  z = rot_w @ x + rot_b (1x1 conv, 3 out ch), BN over (B,H,W) batch stats,
  angles = spatial mean of z_norm, angle = tanh(sum_i angles)*pi/4,
  s = cos(angle); the 3x3x3 FFT-domain weight tensor is phase-rotated by
  exp(-i*beta*G) with beta = 2*pi*s/3, inverse-FFT'd, mid-slice taken ->
  per-sample 3x3 2D kernels; then a batch-as-groups conv2d (pad 1).

Sharding: data-parallel over B across 8 NeuronCores, one sample per core.
Cross-core work: only the BN batch statistics (an AllReduce of 6 floats).

v2 design (single-pass, bf16 at rest):
  - host casts x to bf16; device input is bf16 (halves HBM read bytes and
    DMA packet counts, which dominate the runtime); output is written
    bf16 and upcast to f32 on the host.
  - x is read from HBM exactly ONCE, directly into the 86 resident conv
    strip tiles [128=(c,y-y0), 514] (6-row strips, 8-row window, 1-col
    zero pad each side).
  - BN statistics come from the same strips: a stats lhsT that only
    counts rows p=0..5 of each strip (rows 6i-1..6i+4) sees every image
    row exactly once (out-of-range rows are zeroed).  7 strips' z0 land
    side-by-side in one PSUM tile [126, 512] so the Square/reduce
    post-ops amortize 7x.
  - AllReduce (add) of [S1[i], S2[i]] over the 8 cores (6 floats).
  - scalars -> rotation phase matrix M; w2dT = m_re^T@wtt_re +
    m_imn^T@wtt_im lands directly as [9=(uv), 256=(c,o)] (no PE
    transpose); cast to bf16; 54 tiny DMAs scatter the 3 banded conv
    lhsT matrices with columns ordered (o, ys).
  - conv: 86 strips x 3 accumulating bf16 matmuls (K=128, M=96=(o,ys),
    N=512); psum -> bf16 -> HBM with descriptors emitted in
    HBM-contiguous (o-major) order for DGE packet aggregation.
"""

import sys

for _p in ("/opt/trn_rl_repo", "/root/.axon_site/_ro/trn_rl_repo"):
    if _p not in sys.path:
        sys.path.insert(0, _p)

import ml_dtypes
import numpy as np

import concourse.bacc as bacc
from concourse.ap import AP
import concourse.mybir as mybir
import concourse.tile as tile
from concourse.bass_utils import run_bass_kernel_spmd

F32 = mybir.dt.float32
BF16 = mybir.dt.bfloat16
AF = mybir.ActivationFunctionType
ALU = mybir.AluOpType
AX = mybir.AxisListType

B, C, O, K, H, W = 8, 16, 16, 3, 512, 512
NCORES = 8
HWPIX = H * W                    # 262144
NPIX = B * HWPIX                 # 2097152
BN_EPS = 1e-5
WPAD = W + 2                     # row layout: [0pad, x0..x511, 0pad]
SROWS = 6                        # output rows per conv strip
NSTRIP = (H + SROWS - 1) // SROWS  # 86 (last strip has 2 valid rows)
GS = 4                           # strips per stats group (32 psum rows each)
NGROUP = (NSTRIP + GS - 1) // GS   # 22 (last group has 2 strips)
PI = float(np.pi)


def _consts():
    """Host-precomputed, input-independent constants (baked into the NEFF)."""
    g = np.array([0, 1, -1], np.int64)          # 3*fftfreq(3)
    j1, j2, j3 = np.meshgrid(np.arange(3), np.arange(3), np.arange(3),
                             indexing="ij")
    G = (g[j1] + g[j2] + g[j3]).reshape(27)     # in [-3, 3]

    sel_cos = np.zeros((4, 27), np.float32)
    sel_sin = np.zeros((4, 27), np.float32)
    for j in range(27):
        a = abs(G[j])
        sel_cos[a, j] = 1.0
        if G[j] != 0:
            sgn = float(np.sign(G[j]))
            # sin_t[2] holds sin(2b - pi) = -sin(2b): fold the flip in here
            sel_sin[a, j] = -sgn if a == 2 else sgn

    u = np.arange(3)[None, :, None]
    v = np.arange(3)[None, None, :]
    cang = (2.0 * np.pi / 3.0) * (j1.reshape(27, 1, 1) * 1
                                  + j2.reshape(27, 1, 1) * u
                                  + j3.reshape(27, 1, 1) * v)
    cang = cang.reshape(27, 9)
    cosC = (np.cos(cang) / 27.0).astype(np.float32)
    sinC = (np.sin(cang) / 27.0).astype(np.float32)

    kconst = ((2.0 * np.pi / 3.0) * np.arange(4)).reshape(4, 1).astype(np.float32)
    shift_s = np.array([0.0, 0.0, -np.pi, -2.0 * np.pi], np.float32).reshape(4, 1)
    sigma = np.array([1.0, -1.0, -1.0, 1.0], np.float32).reshape(4, 1)
    tau = np.array([np.pi / 2, np.pi / 2, np.pi / 2, -1.5 * np.pi],
                   np.float32).reshape(4, 1)

    # fold (j in GS, i in 3, p in 6) psum-partition stats down to i;
    # each strip's block is padded to 32 psum partitions (PE base-partition
    # granularity), cols 18..31 are zero
    foldS = np.zeros((GS * 32, 3), np.float32)
    for j in range(GS):
        for i in range(3):
            for p in range(6):
                foldS[j * 32 + i * 6 + p, i] = 1.0
    return dict(sel_cos=sel_cos, sel_sin=sel_sin, cosC=cosC, sinC=sinC,
                kconst=kconst, shift_s=shift_s, sigma=sigma, tau=tau,
                foldS=foldS)


def build_nc():
    nc = bacc.Bacc("TRN2", target_bir_lowering=False, debug=False,
                   num_devices=NCORES)

    # x pre-expanded on host into the SBUF strip layout: row (c*8+p),
    # col (i*WPAD + w) = x_padded[c, 6i+p-1, w-1] (zeros outside the image)
    x_in = nc.dram_tensor("x", [128, NSTRIP * WPAD], BF16,
                          kind="ExternalInput")
    wfr_in = nc.dram_tensor("w_fft_real", [O, C, K, K, K], F32,
                            kind="ExternalInput")
    wfi_in = nc.dram_tensor("w_fft_imag", [O, C, K, K, K], F32,
                            kind="ExternalInput")
    rotw_in = nc.dram_tensor("rot_w", [3, C], F32, kind="ExternalInput")
    gam_in = nc.dram_tensor("bn_gamma", [3], F32, kind="ExternalInput")
    bet_in = nc.dram_tensor("bn_beta", [3], F32, kind="ExternalInput")
    out_t = nc.dram_tensor("out", [O, H, W], BF16, kind="ExternalOutput")

    cc_in = nc.dram_tensor("cc_in", [1, 8], F32)    # internal bounce
    cc_out = nc.dram_tensor("cc_out", [1, 8], F32)
    w2d_dram = nc.dram_tensor("w2d_dram", [9, 256], BF16)  # w2dT bounce

    cst = _consts()
    c_selcos = nc.inline_tensor(cst["sel_cos"], "c_selcos")
    c_selsin = nc.inline_tensor(cst["sel_sin"], "c_selsin")
    c_cosC = nc.inline_tensor(cst["cosC"], "c_cosC")
    c_sinC = nc.inline_tensor(cst["sinC"], "c_sinC")
    c_kconst = nc.inline_tensor(cst["kconst"], "c_kconst")
    c_shift = nc.inline_tensor(cst["shift_s"], "c_shift")
    c_sigma = nc.inline_tensor(cst["sigma"], "c_sigma")
    c_tau = nc.inline_tensor(cst["tau"], "c_tau")
    c_foldS = nc.inline_tensor(cst["foldS"], "c_foldS")

    with tile.TileContext(nc) as tc:
        with tc.tile_pool(name="persist", bufs=1) as pp:
            lhsT_s = pp.tile([128, 32], BF16)       # stats weights (p<6 only)
            _spacer0 = pp.tile([128, 512], F32)     # shadow-cell spacer
            wtt_re = pp.tile([32, 256], F32)   # rows 27..31 zero (K pad)
            wtt_im = pp.tile([32, 256], F32)
            s1cols = pp.tile([128, NGROUP], F32)
            _spacer1 = pp.tile([128, 512], F32)     # shadow-cell spacer
            s2cols = pp.tile([128, NGROUP], F32)
            _spacer2 = pp.tile([128, 512], F32)     # shadow-cell spacer
            ssum = pp.tile([128, 2], F32)
            loc_s = pp.tile([1, 8], F32)
            tot_s = pp.tile([1, 8], F32)
            gam_sb = pp.tile([1, 3], F32)
            bet_sb = pp.tile([1, 3], F32)
            selcos_sb = pp.tile([32, 32], F32)  # [4,27] + zero pad
            selsin_sb = pp.tile([32, 32], F32)
            cosC_sb = pp.tile([27, 9], F32)
            sinC_sb = pp.tile([27, 9], F32)
            kconst_sb = pp.tile([4, 1], F32)
            shift_sb = pp.tile([4, 1], F32)
            sigma_sb = pp.tile([4, 1], F32)
            tau_sb = pp.tile([4, 1], F32)
            foldS_sb = pp.tile([128, 32], F32)  # cols 3..31 zero
            rotw_f32 = pp.tile([16, 3], F32)
            rotw_bf = pp.tile([16, 3], BF16)
            pw_sb = pp.tile([9, 256], F32)
            # all three dx-blocks in one tensor: concurrent scatter writes
            # from different DMA queues stay within one tensor (the race
            # checker tolerates same-tensor disjoint writes), padded to a
            # full 2KB shadow cell and spacer-isolated from neighbors
            _spacer3 = pp.tile([128, 512], F32)
            lhsT_all = pp.tile([128, 1024], BF16, name="lhsT_all")
            lhsT_c = [lhsT_all[:, dx * 128:dx * 128 + 128] for dx in range(3)]
            _spacer4 = pp.tile([128, 512], F32)
            sc3 = pp.tile([1, 3], F32, tag="sc3a")   # phase-C temporaries
            sc3b = pp.tile([1, 3], F32, tag="sc3b")
            sc3c = pp.tile([1, 3], F32, tag="sc3c")
            sc1 = pp.tile([1, 1], F32, tag="sc1a")
            sc1b = pp.tile([1, 1], F32, tag="sc1b")
            kb = pp.tile([4, 1], F32, tag="kb")
            s4 = pp.tile([4, 1], F32, tag="s4")
            sin_t = pp.tile([32, 1], F32, tag="sint")   # rows 4+ zero
            cos_t = pp.tile([32, 1], F32, tag="cost")
            bg = pp.tile([27, 2], F32, tag="bg")
            m_re = pp.tile([32, 32], F32, tag="mre")    # [27,9] + zero pad
            m_imn = pp.tile([32, 32], F32, tag="mimn")
            mt1 = pp.tile([27, 9], F32, tag="mt1")
            mt2 = pp.tile([27, 9], F32, tag="mt2")
            eps_sb = pp.tile([1, 1], F32, tag="eps_sb")
            nqpi_sb = pp.tile([1, 1], F32, tag="nqpi_sb")
            hpi_sb = pp.tile([1, 1], F32, tag="hpi_sb")
            nc.vector.memset(wtt_re[:], 0.0)
            nc.vector.memset(wtt_im[:], 0.0)
            nc.vector.memset(selcos_sb[:], 0.0)
            nc.vector.memset(selsin_sb[:], 0.0)
            nc.vector.memset(foldS_sb[:], 0.0)
            nc.vector.memset(sin_t[:], 0.0)
            nc.vector.memset(cos_t[:], 0.0)
            nc.vector.memset(m_re[:], 0.0)
            nc.vector.memset(m_imn[:], 0.0)
            nc.vector.memset(eps_sb[:], BN_EPS)
            nc.vector.memset(nqpi_sb[:], -PI / 4.0)
            nc.vector.memset(hpi_sb[:], PI / 2.0)

            # all strips in one resident tensor; strip i at cols
            # [i*WPAD, (i+1)*WPAD)
            stripall = pp.tile([128, NSTRIP * WPAD], BF16, name="stripall")

            def strip(i):
                return stripall[:, i * WPAD:(i + 1) * WPAD]

            # ---- one-time setup ----
            nc.gpsimd.dma_start(selcos_sb[0:4, 0:27], c_selcos.ap())
            nc.gpsimd.dma_start(selsin_sb[0:4, 0:27], c_selsin.ap())
            nc.gpsimd.dma_start(cosC_sb[:], c_cosC.ap())
            nc.gpsimd.dma_start(sinC_sb[:], c_sinC.ap())
            nc.gpsimd.dma_start(kconst_sb[:], c_kconst.ap())
            nc.gpsimd.dma_start(shift_sb[:], c_shift.ap())
            nc.gpsimd.dma_start(sigma_sb[:], c_sigma.ap())
            nc.gpsimd.dma_start(tau_sb[:], c_tau.ap())
            nc.gpsimd.dma_start(foldS_sb[:, 0:3], c_foldS.ap())
            nc.gpsimd.dma_start(gam_sb[:], gam_in.ap().unsqueeze(0))
            nc.gpsimd.dma_start(bet_sb[:], bet_in.ap().unsqueeze(0))
            # transposed FFT weights: [27, (c,o)] so the w2dT matmul output
            # free dim is (c,o). One DMA per channel (3-dim DMA limit).
            wtt_src_re = wfr_in.ap().rearrange("o c a b d -> c (a b d) o")
            wtt_src_im = wfi_in.ap().rearrange("o c a b d -> c (a b d) o")
            for c in range(C):
                nc.gpsimd.dma_start(wtt_re[0:27, c * O:(c + 1) * O],
                                    wtt_src_re[c])
                nc.gpsimd.dma_start(wtt_im[0:27, c * O:(c + 1) * O],
                                    wtt_src_im[c])
            # stats lhsT: lhsT_s[(c*8+p), (i*6+p)] = rot_w[i, c] for p < 6
            nc.gpsimd.dma_start(rotw_f32[:], rotw_in.ap().rearrange("i c -> c i"))
            nc.vector.tensor_copy(rotw_bf[:], rotw_f32[:])
            nc.vector.memset(lhsT_s[:], 0.0)
            for p in range(6):
                nc.sync.dma_start(lhsT_s[p::8, p:18:6], rotw_bf[:])
            nc.vector.memset(loc_s[:], 0.0)
            nc.vector.memset(s1cols[:], 0.0)
            nc.vector.memset(s2cols[:], 0.0)
            load_engs = [nc.sync, nc.gpsimd, nc.sync, nc.scalar]

            def load_group(q, eng):
                # one 2-dim DMA per group: contiguous ~4KB per partition line
                i0 = GS * q
                k = min(GS, NSTRIP - i0)
                c0, c1 = i0 * WPAD, (i0 + k) * WPAD
                eng.dma_start(stripall[:, c0:c1], x_in.ap()[:, c0:c1])

            # ---- phase A: load strips + z0 statistics from them ----
            with (
                tc.tile_pool(name="pa_psum", bufs=4, space="PSUM") as pza,
                tc.tile_pool(name="pa_scr", bufs=2) as psc,
            ):
                for q in range(NGROUP):
                    n_in_g = min(GS, NSTRIP - q * GS)
                    mrows = 32 * n_in_g
                    load_group(q, load_engs[q % 4])
                    z0 = pza.tile([128, W], F32, tag="z0")
                    for j in range(n_in_g):
                        i = q * GS + j
                        nc.tensor.matmul(z0[32 * j:32 * (j + 1), :], lhsT_s[:],
                                         strip(i)[:, 1:1 + W],
                                         start=True, stop=True,
                                         tile_position=(0, 32 * j))
                    scr = psc.tile([128, W], BF16, tag="scr")
                    nc.scalar.activation(scr[0:mrows, :], z0[0:mrows, :],
                                         AF.Square,
                                         accum_out=s2cols[0:mrows, q:q + 1])
                    nc.vector.reduce_sum(s1cols[0:mrows, q:q + 1],
                                         z0[0:mrows, :], axis=AX.X)

            # ---- phase B: fold + AllReduce ----
            with tc.tile_pool(name="pb_psum", bufs=1, space="PSUM") as pzb:
                nc.vector.reduce_sum(ssum[:, 0:1], s1cols[:], axis=AX.X)
                nc.vector.reduce_sum(ssum[:, 1:2], s2cols[:], axis=AX.X)
                pf = pzb.tile([32, 2], F32, tag="pf")
                nc.tensor.matmul(pf[:], foldS_sb[:], ssum[:],
                                 start=True, stop=True)
                pf_sb = pp.tile([3, 2], F32, tag="pf_sb")
                nc.vector.tensor_copy(pf_sb[:], pf[0:3, :])
                # interleaved (S1[0],S2[0],S1[1],S2[1],S1[2],S2[2])
                nc.sync.dma_start(loc_s[:, 0:6], pf_sb[:])
                nc.sync.dma_start(cc_in.ap(), loc_s[:])
                nc.gpsimd.collective_compute(
                    "AllReduce", ALU.add,
                    replica_groups=[list(range(NCORES))],
                    ins=[cc_in.ap()], outs=[cc_out.ap()])
                nc.sync.dma_start(tot_s[:], cc_out.ap())

                # ---- phase C: scalars -> rotation -> w2dT -> conv lhsT ----
                t1 = tot_s[:, 0:6:2]     # sum z0   (over batch)
                t2 = tot_s[:, 1:6:2]     # sum z0^2 (over batch)
                nc.vector.tensor_scalar_mul(sc3[:], t1, 1.0 / NPIX)   # m1
                nc.vector.tensor_scalar_mul(sc3b[:], t2, 1.0 / NPIX)  # e2
                nc.vector.tensor_tensor(sc3c[:], sc3[:], sc3[:], op=ALU.mult)
                nc.vector.tensor_tensor(sc3b[:], sc3b[:], sc3c[:],
                                        op=ALU.subtract)              # var
                nc.scalar.activation(sc3b[:], sc3b[:], AF.Sqrt,
                                     bias=eps_sb[:])
                nc.vector.reciprocal(sc3b[:], sc3b[:])                # rsqrt
                nc.vector.tensor_tensor(sc3b[:], sc3b[:], gam_sb[:],
                                        op=ALU.mult)                  # inv
                nc.vector.tensor_scalar_mul(sc3c[:], loc_s[:, 0:6:2],
                                            1.0 / HWPIX)              # s1h
                nc.vector.tensor_tensor(sc3c[:], sc3c[:], sc3[:],
                                        op=ALU.subtract)              # diff
                nc.vector.tensor_tensor(sc3c[:], sc3c[:], sc3b[:],
                                        op=ALU.mult)
                nc.vector.tensor_tensor(sc3c[:], sc3c[:], bet_sb[:],
                                        op=ALU.add)                   # angles
                nc.vector.reduce_sum(sc1[:], sc3c[:], axis=AX.X)      # a
                nc.scalar.activation(sc1b[:], sc1[:], AF.Tanh)
                # s = cos(tanh(a)*pi/4) = sin(pi/2 - (pi/4)*tanh(a))
                nc.scalar.activation(sc1[:], sc1b[:], AF.Sin,
                                     scale=nqpi_sb[:], bias=hpi_sb[:])
                nc.gpsimd.partition_broadcast(s4[:], sc1[:])
                nc.vector.tensor_tensor(kb[:], kconst_sb[:], s4[:],
                                        op=ALU.mult)                  # k*beta
                nc.scalar.activation(sin_t[0:4, :], kb[:], AF.Sin,
                                     bias=shift_sb[:])
                nc.scalar.activation(cos_t[0:4, :], kb[:], AF.Sin,
                                     scale=sigma_sb[:], bias=tau_sb[:])
                pg0 = pzb.tile([32, 1], F32, tag="pg0")
                pg1 = pzb.tile([32, 1], F32, tag="pg1")
                nc.tensor.matmul(pg0[:], selcos_sb[:], cos_t[:],
                                 start=True, stop=True)
                nc.tensor.matmul(pg1[:], selsin_sb[:], sin_t[:],
                                 start=True, stop=True)
                nc.vector.tensor_copy(bg[:, 0:1], pg0[0:27, :])
                nc.vector.tensor_copy(bg[:, 1:2], pg1[0:27, :])
                # M_re = cosC*cbG + sinC*sbG ; M_imn = cosC*sbG - sinC*cbG
                nc.vector.tensor_scalar(mt1[:], cosC_sb[:], bg[:, 0:1], None,
                                        op0=ALU.mult)
                nc.vector.tensor_scalar(mt2[:], sinC_sb[:], bg[:, 1:2], None,
                                        op0=ALU.mult)
                nc.vector.tensor_tensor(m_re[0:27, 0:9], mt1[:], mt2[:],
                                        op=ALU.add)
                nc.vector.tensor_scalar(mt1[:], cosC_sb[:], bg[:, 1:2], None,
                                        op0=ALU.mult)
                nc.vector.tensor_scalar(mt2[:], sinC_sb[:], bg[:, 0:1], None,
                                        op0=ALU.mult)
                nc.vector.tensor_tensor(m_imn[0:27, 0:9], mt1[:], mt2[:],
                                        op=ALU.subtract)
                # w2dT[uv, (c,o)] = m_re^T@wtt_re + m_imn^T@wtt_im
                pw = pzb.tile([32, 256], F32, tag="pw")
                nc.tensor.matmul(pw[:], m_re[:, 0:32], wtt_re[:],
                                 start=True, stop=False)
                nc.tensor.matmul(pw[:], m_imn[:, 0:32], wtt_im[:],
                                 start=False, stop=True)
                nc.vector.tensor_copy(pw_sb[:], pw[0:9, :])
                # bounce w2dT through DRAM (bf16 cast via gpsimd DMA) so the
                # 54 scatters below can fan one row out to 16 partitions
                nc.gpsimd.dma_start(w2d_dram.ap(), pw_sb[:])
                # scatter into conv lhsT: lhsT_c[dx][(c*8+yw), (o*6+ys)]
                #   = w2d[c, o, dy=yw-ys, dx]
                nc.vector.memset(lhsT_all[:], 0.0)
                _n = 0
                for dx in range(3):
                    for dy in range(3):
                        src = w2d_dram.ap()[dy * 3 + dx] \
                            .rearrange("(c o) -> c o", o=O)
                        for ys in range(SROWS):
                            yw = ys + dy
                            eng = (nc.sync, nc.scalar, nc.gpsimd)[_n % 3]
                            eng.dma_start(lhsT_c[dx][yw::8, ys:96:6], src)
                            _n += 1

            # ---- phase D: the batch-as-groups conv ----
            HB = 43                 # strips per output half
            osb_big = [pp.tile([96, HB * W], BF16, name=f"osb_big{h}")
                       for h in range(2)]
            with tc.tile_pool(name="pd_psum", bufs=8, space="PSUM") as pcv:
                for i in range(NSTRIP):
                    pc = pcv.tile([128, W], F32, tag="pc")
                    for dx in range(3):
                        nc.tensor.matmul(pc[:], lhsT_c[dx],
                                         strip(i)[:, dx:dx + W],
                                         start=(dx == 0), stop=(dx == 2))
                    h, j = divmod(i, HB)
                    oseg = osb_big[h][:, j * W:(j + 1) * W]
                    # PSUM has no DMA route; alternate evac engines
                    if i % 2 == 0:
                        nc.scalar.activation(oseg, pc[0:96, :], AF.Copy)
                    else:
                        nc.vector.tensor_copy(oseg, pc[0:96, :])
                    if i == HB - 1 or i == NSTRIP - 1:
                        # drain half h: one DMA per ys row-phase
                        y0 = (i - HB + 1) * SROWS if i == HB - 1 else HB * SROWS
                        i0 = 0 if i == HB - 1 else HB
                        for ys in range(SROWS):
                            # valid strips for this ys in this half
                            kv = sum(1 for ii in range(i0, i + 1)
                                     if SROWS * ii + ys < H)
                            if kv == 0:
                                continue
                            y_first = SROWS * i0 + ys
                            y_stop = y_first + (kv - 1) * SROWS + 1
                            dst = out_t.ap()[:, y_first:y_stop:SROWS, :]
                            srcv = osb_big[h][ys::SROWS, 0:kv * W] \
                                .rearrange("p (j w) -> p j w", w=W)
                            eng = (nc.sync, nc.gpsimd)[ys % 2]
                            eng.dma_start(dst, srcv)

    nc.compile()
    return nc


_NC_CACHE = {}


def _get_nc(key=0):
    if key not in _NC_CACHE:
        _NC_CACHE[key] = build_nc()
    return _NC_CACHE[key]


def _install_ntff_hook():
    """Shim the missing antenv.axon_hooks so trace=True can profile."""
    try:
        import antenv.axon_hooks  # noqa: F401
        return
    except ImportError:
        pass
    import types

    import antenv

    if "/root/.axon_site" not in sys.path:
        sys.path.insert(0, "/root/.axon_site")
    from trn_agent_boot.trn_boot import _ntff_profile_via_ctypes

    hook = _ntff_profile_via_ctypes("/opt/axon/libaxon_pjrt.so")
    m = types.ModuleType("antenv.axon_hooks")
    holder = {"h": hook}
    m.get_axon_ntff_profile_hook = lambda: holder["h"]
    m.set_axon_ntff_profile_hook = lambda h: holder.__setitem__("h", h)
    sys.modules["antenv.axon_hooks"] = m
    antenv.axon_hooks = m


def run_kernel(inputs, trace=False, trace_kwargs=None):
    nc = _get_nc()
    if trace:
        try:
            _install_ntff_hook()
        except Exception as e:
            print(f"ntff hook install failed ({e}); tracing may be skipped")
    x = np.asarray(inputs["x"], np.float32).astype(ml_dtypes.bfloat16)
    xp = np.zeros((B, C, H + 6, WPAD), ml_dtypes.bfloat16)
    xp[:, :, 1:1 + H, 1:1 + W] = x
    # expand into the device strip layout [128=(c,p), 86*514=(i,w)]
    xs = np.empty((B, 128, NSTRIP * WPAD), ml_dtypes.bfloat16)
    for i in range(NSTRIP):
        xs[:, :, i * WPAD:(i + 1) * WPAD] = \
            xp[:, :, SROWS * i:SROWS * i + 8, :].reshape(B, 128, WPAD)
    shared = {
        "w_fft_real": np.ascontiguousarray(inputs["w_fft_real"], np.float32),
        "w_fft_imag": np.ascontiguousarray(inputs["w_fft_imag"], np.float32),
        "rot_w": np.ascontiguousarray(inputs["rot_w"], np.float32),
        "bn_gamma": np.ascontiguousarray(inputs["bn_gamma"], np.float32),
        "bn_beta": np.ascontiguousarray(inputs["bn_beta"], np.float32),
    }
    in_maps = [dict(x=np.ascontiguousarray(xs[b]), **shared) for b in range(B)]
    kw = {}
    if trace:
        kw = dict(trace=True, **(trace_kwargs or {}))
    res = run_bass_kernel_spmd(nc, in_maps, list(range(NCORES)), **kw)
    out = np.stack([res.results[b]["out"] for b in range(B)], axis=0)
    return out.astype(np.float32), res


def kernel(**inputs):
    out, _ = run_kernel(inputs)
    return out
